# revision 5
# baseline (speedup 1.0000x reference)
"""Trainium2 Bass kernel for nn_AttnReadout (segment attention readout), v2.

Computation (reference):
    anchor[b]  = mean of ifeat rows in segment b                  [B, D]
    e[i]       = sigmoid(ifeat @ Wu.T + (anchor @ Wv.T + bv)[seg]) @ we
    alpha      = segment_softmax(e)
    rst[b]     = sum_i alpha[i] * ifeat[i]                        [B, D]
    out        = concat([rst, anchor], axis=1)                    [B, 2D]

Sharding: 2048 segments -> 8 cores x 2 windows of 128 contiguous segments.
Nodes (sorted by segment) padded per-window to T_W tiles of 128 rows.

v2 design vs v1:
  - u = ifeat @ Wu.T computed TRANSPOSED (uT[fo, node]) via fp8 DoubleRow
    matmuls with stationary Wu (K=256 in one mm, 2x stream rate).
  - fv[seg] broadcast to nodes via plain-fp8 matmul with stationary fv and a
    host-precomputed transposed one-hot (ohT8) streamed from HBM: kills the
    per-tile PE transposes and PSUM->SBUF copies of v1.
  - e = we . sigmoid(s) moved from DVE (327ns/tile stt) to PE: per tile two
    N=1 matmuls with lhsT = sigmaT tile, accumulating into an e PSUM column.
  - wsum one-hot is built fused: ohz = (iota==seg)*z in ONE DVE op.
  - counts per segment are compile-time known (host rcnt), anchor/wsum stay
    bf16 (fp8 nat fails the 2e-2 gate; fp8 u+fv passes at ~8e-3).
  - z = exp(e) = sigmoid(e)/sigmoid(-e) per chunk (no ACT table swap).
  - Wu/fv/Wv.T pre-scaled by 16 on host (fp8 range), undone by the ACT
    sigmoid's free affine scale=1/16.
"""

import numpy as np
import ml_dtypes

N = 102400
D = 256
B = 2048
N_CORES = 8
W_PER_CORE = 2
N_WINDOWS = N_CORES * W_PER_CORE  # 16
SEGS_PER_WINDOW = B // N_WINDOWS  # 128
P = 128
BF = ml_dtypes.bfloat16
F8 = ml_dtypes.float8_e4m3


def _apply_tile_patch():
    """Split TileContext's multi-wait tail drain into single-wait drains
    (this walrus build rejects >1 sync wait on a Drain instruction)."""
    import concourse.tile as tile_mod
    from concourse.vector_clock import ScopedClock

    if getattr(tile_mod.TileContext, "_drain_wait_split_patch", False):
        return

    def _patched(self, tick_clock, wait_clock):
        nc = self.nc
        drain_inst = nc.sync.drain()
        wait_clock.add_sem_waits(
            drain_inst.ins, ScopedClock({None: tick_clock.global_clock})
        )
        si = drain_inst.ins.sync_info
        waits = list(si.on_wait) if si is not None else []
        if len(waits) > 1:
            SyncInfo = type(si)
            drain_inst.ins.sync_info = SyncInfo(
                on_wait=[waits[0]], on_update=list(si.on_update)
            )
            for w in waits[1:]:
                extra = nc.sync.drain()
                extra.ins.sync_info = SyncInfo(on_wait=[w], on_update=[])

        nc.all_engine_barrier()
        assert self.sems is not None
        popped = nc._tile_sem_poison_stack.pop()
        assert popped is self._sem_poison
        nc.clear_and_free_semaphores(list(self.sems.allocated().values()))
        nc.all_engine_barrier()

    tile_mod.TileContext._drain_and_barrier = _patched
    tile_mod.TileContext._drain_wait_split_patch = True


def _split_sync_waits(nc, limit=1):
    """Split >limit sync waits per instruction into preceding single-wait
    EventSemaphore carriers on the same engine (walrus build limit)."""
    import concourse.mybir as mybir

    n_new = 0
    for _, bassbb in nc.bb_map.items():
        insts = bassbb.bb.instructions  # live list
        snapshot = list(insts)
        offset = 0
        for pos, inst in enumerate(snapshot):
            si = getattr(inst, "sync_info", None)
            if si is None:
                continue
            waits = list(si.on_wait)
            if len(waits) <= limit:
                continue
            SyncInfo = type(si)
            inst.sync_info = SyncInfo(
                on_wait=waits[:limit], on_update=list(si.on_update))
            carriers = []
            for w in waits[limit:]:
                c = mybir.InstEventSemaphore(
                    name=f"WSPLIT-{nc.next_id()}", ins=[], outs=[])
                c.engine = inst.engine
                c.sync_info = SyncInfo(on_wait=[w], on_update=[])
                carriers.append(c)
            insts[pos + offset:pos + offset] = carriers
            offset += len(carriers)
            n_new += len(carriers)
    return n_new


def _build(T_W, repeat=1, loop_repeat=None):
    """Build the single-core SPMD Bass program; T_W must be a multiple of 4."""
    import contextlib
    import concourse.bass as bass
    import concourse.mybir as mybir
    from concourse.tile import TileContext

    _apply_tile_patch()

    f32 = mybir.dt.float32
    bf16 = mybir.dt.bfloat16
    fp8 = mybir.dt.float8e4
    Alu = mybir.AluOpType
    Act = mybir.ActivationFunctionType
    DR = mybir.MatmulPerfMode.DoubleRow

    assert T_W % 4 == 0
    CH = T_W // 4           # tiles per nat DMA chunk (4 chunks per window)
    UC = T_W // 4           # u-chunks per window (4 tiles = 512 nodes each)
    NT = W_PER_CORE * T_W

    nc = bass.Bass("TRN2", num_devices=N_CORES)

    nat_dram = nc.dram_tensor("natp", [P, NT, D + 1], bf16, kind="ExternalInput")
    if8_dram = nc.dram_tensor("ift8p", [P, NT * P, 2], fp8, kind="ExternalInput")
    ohT_dram = nc.dram_tensor("ohT8p", [P, NT * P], fp8, kind="ExternalInput")
    cbf_dram = nc.dram_tensor("cbf", [P, 770], bf16, kind="ExternalInput")
    cf32_dram = nc.dram_tensor("cf32", [P, D + 2 + NT + P], f32,
                               kind="ExternalInput")
    wu8_dram = nc.dram_tensor("wu8p", [P, 512], fp8, kind="ExternalInput")
    out_dram = nc.dram_tensor("out", [W_PER_CORE, P, 2 * D], f32,
                              kind="ExternalOutput")

    with TileContext(nc) as tc:
        with contextlib.ExitStack() as ctx:
            const_pool = ctx.enter_context(tc.tile_pool(name="const", bufs=1))
            nat_pool = ctx.enter_context(tc.tile_pool(name="nat", bufs=1))
            u8_pool = ctx.enter_context(tc.tile_pool(name="u8", bufs=1))
            ohw_pool = ctx.enter_context(tc.tile_pool(name="ohw", bufs=6))
            sgT_pool = ctx.enter_context(tc.tile_pool(name="sgT", bufs=4))
            wnd_pool = ctx.enter_context(tc.tile_pool(name="wnd", bufs=2))
            col_pool = ctx.enter_context(tc.tile_pool(name="col", bufs=8))
            zch_pool = ctx.enter_context(tc.tile_pool(name="zch", bufs=8))
            ups_pool = ctx.enter_context(
                tc.tile_pool(name="ups", bufs=3, space="PSUM"))
            sps_pool = ctx.enter_context(
                tc.tile_pool(name="sps", bufs=1, space="PSUM"))

            # ---- constants (packed by dtype: 3 DMAs) ----
            cbf_sb = const_pool.tile([P, 770], bf16, name="cbf_sb", tag="cbf_sb")
            nc.sync.dma_start(cbf_sb[:], cbf_dram[:])
            cf32_sb = const_pool.tile([P, D + 2 + NT + P], f32,
                                      name="cf32_sb", tag="cf32_sb")
            nc.sync.dma_start(cf32_sb[:], cf32_dram[:])
            wu8f_sb = const_pool.tile([P, 512], fp8, name="wu8f_sb",
                                      tag="wu8f_sb")
            nc.sync.dma_start(wu8f_sb[:], wu8_dram[:])
            wu8_sb = wu8f_sb[:].rearrange("p (h k m) -> p h k m", h=2, k=2)
            wvT_sb = cbf_sb[:, 0:512].rearrange("p (k d) -> p k d", k=2)
            web_sb = cbf_sb[:, 512:514]
            idb_sb = cbf_sb[:, 514:642]
            iota_sb = cbf_sb[:, 642:770]
            bvb_sb = cf32_sb[:, 0:D]
            rcnt_sb = cf32_sb[:, D:D + 2]
            seg_sb = cf32_sb[:, D + 2:D + 2 + NT]
            idf_sb = cf32_sb[:, D + 2 + NT:D + 2 + NT + P]

            def emit_loads(rep, w):
                st = {}
                nat_ch = {}
                NH = T_W // 2
                for cl in range(2):
                    c = 2 * w + cl
                    natc = nat_pool.tile([P, NH, D + 1], bf16,
                                         name=f"natc{rep}_{c}", tag="natc",
                                         bufs=4)
                    nc.sync.dma_start(natc[:], nat_dram[:, c * NH:(c + 1) * NH, :])
                    nat_ch[cl] = natc
                st["nat_ch"] = nat_ch

                # if8/ohT in two window-half DMAs (u-chunk aligned)
                hsplit = (UC // 2) * 512          # nodes in first half
                wbase = w * T_W * P
                wn = T_W * P
                if8_hv = []
                ohT_hv = []
                pieces = ((0, 512), (512, hsplit), (hsplit, wn))
                for hf, (o0, o1) in enumerate(pieces):
                    if8h = u8_pool.tile([P, o1 - o0, 2], fp8,
                                        name=f"if8h{rep}_{w}_{hf}", tag=f"if8c{hf}",
                                        bufs=4)
                    nc.sync.dma_start(if8h[:],
                                      if8_dram[:, wbase + o0:wbase + o1, :])
                    if8_hv.append((o0, if8h))
                    ohTh = u8_pool.tile([P, o1 - o0], fp8,
                                        name=f"ohTh{rep}_{w}_{hf}", tag=f"ohTc{hf}",
                                        bufs=4)
                    nc.sync.dma_start(ohTh[:],
                                      ohT_dram[:, wbase + o0:wbase + o1])
                    ohT_hv.append((o0, ohTh))
                st["if8_hv"] = if8_hv
                st["ohT_hv"] = ohT_hv
                st["hsplit"] = hsplit
                return st

            def emit_pass1(rep, w, st):
                nat_ch = st["nat_ch"]

                NH = T_W // 2

                def nat_t(t):
                    return nat_ch[t // NH][:, t % NH, :]

                st["nat_t"] = nat_t
                # one-hot chunk schedule: big chunks early, 4-tile granules
                # near the window end (to shrink the serial wsum tail)
                zchunks = []
                rem = T_W
                while rem > 20:
                    zchunks.append(16)
                    rem -= 16
                while rem > 0:
                    zchunks.append(4)
                    rem -= 4
                st["zchunks"] = zchunks
                anchor_ps = sps_pool.tile([P, D + 1], f32,
                                          name=f"anc_ps{rep}_{w}",
                                          tag="anchor_ps", bufs=1)
                ohw_cl = {}
                c0 = 0
                for ci, csz in enumerate(zchunks):
                    g0 = w * T_W + c0
                    ohwc = ohw_pool.tile([P, csz, P], bf16,
                                         name=f"ohwc{rep}_{w}_{ci}",
                                         tag=f"ohwc{csz}",
                                         bufs=(5 if csz == 16 else 12))
                    nc.vector.tensor_tensor(
                        ohwc[:],
                        seg_sb[:, g0:g0 + csz, None].broadcast_to([P, csz, P]),
                        iota_sb[:, None, :].broadcast_to([P, csz, P]),
                        Alu.is_equal)
                    ohw_cl[ci] = ohwc
                    for tl in range(csz):
                        t = c0 + tl
                        nc.tensor.matmul(anchor_ps[:], ohwc[:, tl, :], nat_t(t),
                                         start=(t == 0), stop=(t == T_W - 1))
                    c0 += csz
                st["ohw_cl"] = ohw_cl

                out_sb = wnd_pool.tile([P, 2 * D], f32, name=f"osb{rep}_{w}",
                                       tag="out_sb")
                nc.vector.tensor_scalar(out_sb[:, D:2 * D], anchor_ps[:, 0:D],
                                        rcnt_sb[:, w:w + 1], None, Alu.mult)
                st["out_sb"] = out_sb

                # fv8 = fp8((anchor @ Wv.T)*16 + bv*16); transpose scratch and
                # fv output share one PSUM bank (f32 transposes from out_sb)
                tfv = sps_pool.tile([P, 4 * P], f32, name=f"tfv{rep}_{w}",
                                    tag="tfv", bufs=1)
                for db in range(2):
                    nc.tensor.transpose(tfv[:, db * P:(db + 1) * P],
                                        out_sb[:, D + db * P:D + (db + 1) * P],
                                        idf_sb[:])
                anchT = wnd_pool.tile([P, 2, P], bf16, name=f"anchT{rep}_{w}",
                                      tag="anchT")
                nc.any.tensor_copy(anchT[:], tfv[:, 0:2 * P])
                for db in range(2):
                    nc.tensor.matmul(tfv[:, 2 * P:4 * P], anchT[:, db, :],
                                     wvT_sb[:, db, :],
                                     start=(db == 0), stop=(db == 1))
                fv8 = wnd_pool.tile([P, D], fp8, name=f"fv8{rep}_{w}", tag="fv8")
                nc.vector.tensor_tensor(fv8[:], tfv[:, 2 * P:4 * P], bvb_sb[:],
                                        Alu.add)
                st["fv8"] = fv8
                return st

            def emit_pass23(rep, w, st, interject=None):
                nat_t = st["nat_t"]
                ohw_cl = st["ohw_cl"]
                fv8 = st["fv8"]
                out_sb = st["out_sb"]
                hsplit = st["hsplit"]
                if8_hv = st["if8_hv"]
                ohT_hv = st["ohT_hv"]

                def _pick(hv, n0):
                    for o0, tile in reversed(hv):
                        if n0 >= o0:
                            return o0, tile
                    raise AssertionError

                def if8_sl(uc):
                    n0 = uc * 512
                    o0, tile = _pick(if8_hv, n0)
                    return tile[:, n0 - o0:n0 - o0 + 512, :]

                def ohT_sl(uc):
                    n0 = uc * 512
                    o0, tile = _pick(ohT_hv, n0)
                    return tile[:, n0 - o0:n0 - o0 + 512]

                wsum_ps = sps_pool.tile([P, D + 1], f32,
                                        name=f"wsum{rep}_{w}", tag="wsum_ps",
                                        bufs=1)
                e_ps = sps_pool.tile([P, T_W], f32, name=f"e_ps{rep}_{w}",
                                     tag="e_ps", bufs=1)
                z_win = wnd_pool.tile([P, T_W], f32, name=f"zw{rep}_{w}",
                                      tag="z_win")

                zchunks = st["zchunks"]
                zstart = [sum(zchunks[:i]) for i in range(len(zchunks))]
                # chunk ci ready after u-chunk (zstart+csz-1)//4
                zc_after = {}
                for ci, csz in enumerate(zchunks):
                    zc_after.setdefault((zstart[ci] + csz - 1) // 4,
                                        []).append(ci)

                def z_wsum_chunk(ci):
                    csz = zchunks[ci]
                    c0 = zstart[ci]
                    c1 = c0 + csz
                    sp = zch_pool.tile([P, csz], f32, name=f"sp{rep}_{w}_{ci}",
                                       tag=f"zchs{csz}")
                    nc.scalar.activation(sp[:], e_ps[:, c0:c1], Act.Sigmoid)
                    sn = zch_pool.tile([P, csz], f32, name=f"sn{rep}_{w}_{ci}",
                                       tag=f"zchn{csz}")
                    nc.scalar.activation(sn[:], e_ps[:, c0:c1], Act.Sigmoid,
                                         scale=-1.0)
                    rn = zch_pool.tile([P, csz], f32, name=f"rn{rep}_{w}_{ci}",
                                       tag=f"zchr{csz}")
                    nc.vector.reciprocal(rn[:], sn[:])
                    nc.vector.tensor_tensor(z_win[:, c0:c1], sp[:], rn[:],
                                            Alu.mult)
                    ohzc = ohw_pool.tile([P, csz, P], bf16,
                                         name=f"ohzc{rep}_{w}_{ci}",
                                         tag=f"ohzc{csz}",
                                         bufs=(3 if csz == 16 else 5))
                    nc.vector.tensor_tensor(
                        ohzc[:], ohw_cl[ci][:],
                        z_win[:, c0:c1, None].broadcast_to([P, csz, P]),
                        Alu.mult)
                    for tl in range(csz):
                        t = c0 + tl
                        nc.tensor.matmul(wsum_ps[:], ohzc[:, tl, :], nat_t(t),
                                         start=(t == 0), stop=(t == T_W - 1))

                pairs = [(2 * p, 2 * p + 1) for p in range(UC // 2)]
                if UC % 2:
                    pairs.append((UC - 1,))
                for pi, ucs in enumerate(pairs):
                    width = 512 * len(ucs)
                    sgT = sgT_pool.tile([P, 2, width], bf16,
                                        name=f"sgT{rep}_{w}_{pi}",
                                        tag=f"sgT{len(ucs)}",
                                        bufs=(3 if len(ucs) == 2 else 2))
                    for h in range(2):
                        u_ps = ups_pool.tile([P, 1024], f32,
                                             name=f"u_ps{rep}_{w}_{pi}_{h}",
                                             tag="u_ps", bufs=2)
                        for j, uc in enumerate(ucs):
                            sl = slice(j * 512, (j + 1) * 512)
                            nc.tensor.matmul(
                                u_ps[:, sl], wu8_sb[:, h, :, :],
                                if8_sl(uc).rearrange("p n k -> p k n"),
                                start=True, stop=False, perf_mode=DR)
                            nc.tensor.matmul(
                                u_ps[:, sl], fv8[:, h * P:(h + 1) * P],
                                ohT_sl(uc),
                                start=False, stop=True)
                        nc.scalar.activation(sgT[:, h, :], u_ps[:, 0:width],
                                             Act.Sigmoid, scale=0.0625)
                    for j, uc in enumerate(ucs):
                        for t4 in range(4):
                            t = uc * 4 + t4
                            for h in range(2):
                                nc.tensor.matmul(
                                    e_ps[:, t:t + 1],
                                    sgT[:, h, j * 512 + t4 * P:
                                        j * 512 + (t4 + 1) * P],
                                    web_sb[:, h:h + 1],
                                    start=(h == 0), stop=(h == 1))
                        for ci in zc_after.get(uc, ()):
                            z_wsum_chunk(ci)
                        if interject is not None and uc == UC - 5:
                            interject()

                den = col_pool.tile([P, 1], f32, name=f"den{rep}_{w}", tag="col")
                nc.vector.tensor_scalar(den[:], wsum_ps[:, D:D + 1], 1e-30,
                                        None, Alu.max)
                rden = col_pool.tile([P, 1], f32, name=f"rden{rep}_{w}",
                                     tag="col")
                nc.vector.reciprocal(rden[:], den[:])
                nc.vector.tensor_scalar(out_sb[:, 0:D], wsum_ps[:, 0:D],
                                        rden[:], None, Alu.mult)
                nc.sync.dma_start(out_dram[w], out_sb[:])

            def body(rep):
                st0 = emit_loads(rep, 0)
                emit_pass1(rep, 0, st0)
                st1 = emit_loads(rep, 1)

                def _splice():
                    emit_pass1(rep, 1, st1)

                emit_pass23(rep, 0, st0, interject=_splice)
                emit_pass23(rep, 1, st1)

            if loop_repeat is not None:
                with tc.For_i(0, loop_repeat, 1):
                    body("L")
            else:
                for rep in range(repeat):
                    body(rep)

    return nc


def _prepare(ifeat, Wu, Wv, bv, we, seg_ids):
    """Host-side shard + pad + layout. Returns (T_W, in_maps)."""
    ifeat = np.asarray(ifeat, dtype=np.float32)
    Wu = np.asarray(Wu, dtype=np.float32)
    Wv = np.asarray(Wv, dtype=np.float32)
    bv = np.asarray(bv, dtype=np.float32)
    we = np.asarray(we, dtype=np.float32)
    seg_ids = np.asarray(seg_ids)

    bounds = np.searchsorted(
        seg_ids, np.arange(0, B + 1, SEGS_PER_WINDOW), side="left")
    n_w = np.diff(bounds)
    T_W = max(4, int(-(-int(n_w.max()) // P)))
    T_W = ((T_W + 3) // 4) * 4
    NT = W_PER_CORE * T_W

    # wu8p[ki, (h, ko, m)] = Wu[h*128+m, ko*128+ki] * 16
    wu8p = np.ascontiguousarray(
        (Wu * 16.0).reshape(2, P, 2, P).transpose(3, 0, 2, 1)
        .reshape(P, 512)).astype(F8)
    wvT16 = np.ascontiguousarray(Wv.T * 16.0).reshape(2, P, D)
    # cbf[p] = [wvT16 (k d: 512), web2 (2), idb (128), iota (128)]
    cbf = np.concatenate([
        wvT16.transpose(1, 0, 2).reshape(P, 512),
        np.ascontiguousarray(we.reshape(2, P).T),
        np.eye(P, dtype=np.float32),
        np.tile(np.arange(P, dtype=np.float32), (P, 1)),
    ], axis=1).astype(BF)
    bvb16 = np.tile(bv * 16.0, (P, 1)).astype(np.float32)

    in_maps = []
    for c in range(N_CORES):
        nat = np.zeros((NT * P, D + 1), dtype=np.float32)
        nat[:, D] = 1.0
        seg = np.full((NT * P,), 500.0, dtype=np.float32)
        rcnt = np.zeros((P, W_PER_CORE), dtype=np.float32)
        for wl in range(W_PER_CORE):
            w = c * W_PER_CORE + wl
            lo, hi = bounds[w], bounds[w + 1]
            base = wl * T_W * P
            nat[base:base + (hi - lo), 0:D] = ifeat[lo:hi]
            seg_loc = seg_ids[lo:hi].astype(np.int64) - w * SEGS_PER_WINDOW
            seg[base:base + (hi - lo)] = seg_loc.astype(np.float32)
            cnt = np.bincount(seg_loc, minlength=P).astype(np.float32)
            rcnt[:, wl] = 1.0 / np.maximum(cnt, 1.0)
        natb = nat.astype(BF).reshape(NT, P, D + 1)
        natp = np.ascontiguousarray(natb.transpose(1, 0, 2))     # [P, NT, 257]
        x8 = nat[:, 0:D].astype(F8)                              # [n, fi]
        ift8p = np.ascontiguousarray(
            x8.reshape(NT * P, 2, P).transpose(2, 0, 1))         # [ki, n, ko]
        ohT8p = (seg[None, :] ==
                 np.arange(P, dtype=np.float32)[:, None]).astype(F8)
        segp = np.ascontiguousarray(seg.reshape(NT, P).T)        # [P, NT]
        cf32 = np.concatenate([bvb16, rcnt, segp,
                               np.eye(P, dtype=np.float32)],
                              axis=1).astype(np.float32)
        in_maps.append({
            "natp": natp, "ift8p": ift8p, "ohT8p": ohT8p,
            "cbf": cbf, "cf32": cf32, "wu8p": wu8p,
        })
    return T_W, in_maps


_LAST = {}


def _run(ifeat, Wu, Wv, bv, we, seg_ids, trace=False):
    from concourse.bass_utils import run_bass_kernel_spmd

    T_W, in_maps = _prepare(ifeat, Wu, Wv, bv, we, seg_ids)
    nc = _build(T_W)
    _split_sync_waits(nc)
    res = run_bass_kernel_spmd(nc, in_maps, list(range(N_CORES)), trace=trace)
    _LAST["res"] = res
    _LAST["T_W"] = T_W
    _LAST["nc"] = nc
    _LAST["in_maps"] = in_maps

    out = np.empty((B, 2 * D), dtype=np.float32)
    for c in range(N_CORES):
        core_out = res.results[c]["out"]  # [W_PER_CORE, P, 2D]
        for wl in range(W_PER_CORE):
            w = c * W_PER_CORE + wl
            out[w * SEGS_PER_WINDOW:(w + 1) * SEGS_PER_WINDOW, :] = core_out[wl]
    return out


def kernel(ifeat, Wu, Wv, bv, we, seg_ids):
    return _run(ifeat, Wu, Wv, bv, we, seg_ids, trace=False)


# revision 6
# speedup vs baseline: 1.6398x; 1.6398x over previous
"""Trainium2 Bass kernel for nn_AttnReadout (segment attention readout), v2.

Computation (reference):
    anchor[b]  = mean of ifeat rows in segment b                  [B, D]
    e[i]       = sigmoid(ifeat @ Wu.T + (anchor @ Wv.T + bv)[seg]) @ we
    alpha      = segment_softmax(e)
    rst[b]     = sum_i alpha[i] * ifeat[i]                        [B, D]
    out        = concat([rst, anchor], axis=1)                    [B, 2D]

Sharding: 2048 segments -> 8 cores x 2 windows of 128 contiguous segments.
Nodes (sorted by segment) padded per-window to T_W tiles of 128 rows.

v2 design vs v1:
  - u = ifeat @ Wu.T computed TRANSPOSED (uT[fo, node]) via fp8 DoubleRow
    matmuls with stationary Wu (K=256 in one mm, 2x stream rate).
  - fv[seg] broadcast to nodes via plain-fp8 matmul with stationary fv and a
    host-precomputed transposed one-hot (ohT8) streamed from HBM: kills the
    per-tile PE transposes and PSUM->SBUF copies of v1.
  - e = we . sigmoid(s) moved from DVE (327ns/tile stt) to PE: per tile two
    N=1 matmuls with lhsT = sigmaT tile, accumulating into an e PSUM column.
  - wsum one-hot is built fused: ohz = (iota==seg)*z in ONE DVE op.
  - counts per segment are compile-time known (host rcnt), anchor/wsum stay
    bf16 (fp8 nat fails the 2e-2 gate; fp8 u+fv passes at ~8e-3).
  - z = exp(e) = sigmoid(e)/sigmoid(-e) per chunk (no ACT table swap).
  - Wu/fv/Wv.T pre-scaled by 16 on host (fp8 range), undone by the ACT
    sigmoid's free affine scale=1/16.
"""

import numpy as np
import ml_dtypes

N = 102400
D = 256
B = 2048
N_CORES = 8
W_PER_CORE = 2
N_WINDOWS = N_CORES * W_PER_CORE  # 16
SEGS_PER_WINDOW = B // N_WINDOWS  # 128
P = 128
BF = ml_dtypes.bfloat16
F8 = ml_dtypes.float8_e4m3


def _apply_tile_patch():
    """Split TileContext's multi-wait tail drain into single-wait drains
    (this walrus build rejects >1 sync wait on a Drain instruction)."""
    import concourse.tile as tile_mod
    from concourse.vector_clock import ScopedClock

    if getattr(tile_mod.TileContext, "_drain_wait_split_patch", False):
        return

    def _patched(self, tick_clock, wait_clock):
        nc = self.nc
        drain_inst = nc.sync.drain()
        wait_clock.add_sem_waits(
            drain_inst.ins, ScopedClock({None: tick_clock.global_clock})
        )
        si = drain_inst.ins.sync_info
        waits = list(si.on_wait) if si is not None else []
        if len(waits) > 1:
            SyncInfo = type(si)
            drain_inst.ins.sync_info = SyncInfo(
                on_wait=[waits[0]], on_update=list(si.on_update)
            )
            for w in waits[1:]:
                extra = nc.sync.drain()
                extra.ins.sync_info = SyncInfo(on_wait=[w], on_update=[])

        nc.all_engine_barrier()
        assert self.sems is not None
        popped = nc._tile_sem_poison_stack.pop()
        assert popped is self._sem_poison
        nc.clear_and_free_semaphores(list(self.sems.allocated().values()))
        nc.all_engine_barrier()

    tile_mod.TileContext._drain_and_barrier = _patched
    tile_mod.TileContext._drain_wait_split_patch = True


def _split_sync_waits(nc, limit=1):
    """Split >limit sync waits per instruction into preceding single-wait
    EventSemaphore carriers on the same engine (walrus build limit)."""
    import concourse.mybir as mybir

    n_new = 0
    for _, bassbb in nc.bb_map.items():
        insts = bassbb.bb.instructions  # live list
        snapshot = list(insts)
        offset = 0
        for pos, inst in enumerate(snapshot):
            si = getattr(inst, "sync_info", None)
            if si is None:
                continue
            waits = list(si.on_wait)
            if len(waits) <= limit:
                continue
            SyncInfo = type(si)
            inst.sync_info = SyncInfo(
                on_wait=waits[:limit], on_update=list(si.on_update))
            carriers = []
            for w in waits[limit:]:
                c = mybir.InstEventSemaphore(
                    name=f"WSPLIT-{nc.next_id()}", ins=[], outs=[])
                c.engine = inst.engine
                c.sync_info = SyncInfo(on_wait=[w], on_update=[])
                carriers.append(c)
            insts[pos + offset:pos + offset] = carriers
            offset += len(carriers)
            n_new += len(carriers)
    return n_new


def _build(T_W, repeat=1, loop_repeat=None):
    """Build the single-core SPMD Bass program; T_W must be a multiple of 4."""
    import contextlib
    import concourse.bass as bass
    import concourse.mybir as mybir
    from concourse.tile import TileContext

    _apply_tile_patch()

    f32 = mybir.dt.float32
    bf16 = mybir.dt.bfloat16
    fp8 = mybir.dt.float8e4
    Alu = mybir.AluOpType
    Act = mybir.ActivationFunctionType
    DR = mybir.MatmulPerfMode.DoubleRow

    assert T_W % 4 == 0
    CH = T_W // 4           # tiles per nat DMA chunk (4 chunks per window)
    UC = T_W // 4           # u-chunks per window (4 tiles = 512 nodes each)
    NT = W_PER_CORE * T_W

    nc = bass.Bass("TRN2", num_devices=N_CORES)

    nat_dram = nc.dram_tensor("natp", [P, NT, D + 1], bf16, kind="ExternalInput")
    if8_dram = nc.dram_tensor("ift8p", [P, NT * P, 2], fp8, kind="ExternalInput")
    ohT_dram = nc.dram_tensor("ohT8p", [P, NT * P], fp8, kind="ExternalInput")
    cbf_dram = nc.dram_tensor("cbf", [P, 770], bf16, kind="ExternalInput")
    cf32_dram = nc.dram_tensor("cf32", [P, D + 2 + NT + P], f32,
                               kind="ExternalInput")
    wu8_dram = nc.dram_tensor("wu8p", [P, 512], fp8, kind="ExternalInput")
    out_dram = nc.dram_tensor("out", [W_PER_CORE, P, 2 * D], f32,
                              kind="ExternalOutput")

    with TileContext(nc) as tc:
        with contextlib.ExitStack() as ctx:
            const_pool = ctx.enter_context(tc.tile_pool(name="const", bufs=1))
            nat_pool = ctx.enter_context(tc.tile_pool(name="nat", bufs=1))
            u8_pool = ctx.enter_context(tc.tile_pool(name="u8", bufs=1))
            ohw_pool = ctx.enter_context(tc.tile_pool(name="ohw", bufs=6))
            sgT_pool = ctx.enter_context(tc.tile_pool(name="sgT", bufs=4))
            wnd_pool = ctx.enter_context(tc.tile_pool(name="wnd", bufs=2))
            col_pool = ctx.enter_context(tc.tile_pool(name="col", bufs=8))
            zch_pool = ctx.enter_context(tc.tile_pool(name="zch", bufs=8))
            ups_pool = ctx.enter_context(
                tc.tile_pool(name="ups", bufs=3, space="PSUM"))
            sps_pool = ctx.enter_context(
                tc.tile_pool(name="sps", bufs=1, space="PSUM"))

            # ---- constants (packed by dtype: 3 DMAs) ----
            cbf_sb = const_pool.tile([P, 770], bf16, name="cbf_sb", tag="cbf_sb")
            nc.sync.dma_start(cbf_sb[:], cbf_dram[:])
            cf32_sb = const_pool.tile([P, D + 2 + NT + P], f32,
                                      name="cf32_sb", tag="cf32_sb")
            nc.sync.dma_start(cf32_sb[:], cf32_dram[:])
            wu8f_sb = const_pool.tile([P, 512], fp8, name="wu8f_sb",
                                      tag="wu8f_sb")
            nc.sync.dma_start(wu8f_sb[:], wu8_dram[:])
            wu8_sb = wu8f_sb[:].rearrange("p (h k m) -> p h k m", h=2, k=2)
            wvT_sb = cbf_sb[:, 0:512].rearrange("p (k d) -> p k d", k=2)
            web_sb = cbf_sb[:, 512:514]
            idb_sb = cbf_sb[:, 514:642]
            iota_sb = cbf_sb[:, 642:770]
            bvb_sb = cf32_sb[:, 0:D]
            rcnt_sb = cf32_sb[:, D:D + 2]
            seg_sb = cf32_sb[:, D + 2:D + 2 + NT]
            idf_sb = cf32_sb[:, D + 2 + NT:D + 2 + NT + P]

            def emit_loads(rep, w):
                st = {}
                nat_ch = {}
                NH = T_W // 4
                for cl in range(4):
                    c = 4 * w + cl
                    natc = nat_pool.tile([P, NH, D + 1], bf16,
                                         name=f"natc{rep}_{c}", tag="natc",
                                         bufs=8)
                    nc.sync.dma_start(natc[:], nat_dram[:, c * NH:(c + 1) * NH, :])
                    nat_ch[cl] = natc
                st["nat_ch"] = nat_ch

                # if8/ohT in two window-half DMAs (u-chunk aligned)
                hsplit = (UC // 2) * 512          # nodes in first half
                wbase = w * T_W * P
                wn = T_W * P
                if8_hv = []
                ohT_hv = []
                pieces = ((0, 512), (512, hsplit), (hsplit, wn))
                for hf, (o0, o1) in enumerate(pieces):
                    if8h = u8_pool.tile([P, o1 - o0, 2], fp8,
                                        name=f"if8h{rep}_{w}_{hf}", tag=f"if8c{hf}",
                                        bufs=4)
                    nc.sync.dma_start(if8h[:],
                                      if8_dram[:, wbase + o0:wbase + o1, :])
                    if8_hv.append((o0, if8h))
                    ohTh = u8_pool.tile([P, o1 - o0], fp8,
                                        name=f"ohTh{rep}_{w}_{hf}", tag=f"ohTc{hf}",
                                        bufs=4)
                    nc.sync.dma_start(ohTh[:],
                                      ohT_dram[:, wbase + o0:wbase + o1])
                    ohT_hv.append((o0, ohTh))
                st["if8_hv"] = if8_hv
                st["ohT_hv"] = ohT_hv
                st["hsplit"] = hsplit
                return st

            def emit_pass1(rep, w, st):
                nat_ch = st["nat_ch"]

                NH = T_W // 4

                def nat_t(t):
                    return nat_ch[t // NH][:, t % NH, :]

                st["nat_t"] = nat_t
                # one-hot chunk schedule: big chunks early, 4-tile granules
                # near the window end (to shrink the serial wsum tail)
                zchunks = []
                rem = T_W
                while rem > 20:
                    zchunks.append(16)
                    rem -= 16
                while rem > 0:
                    zchunks.append(4)
                    rem -= 4
                st["zchunks"] = zchunks
                anchor_ps = sps_pool.tile([P, D + 1], f32,
                                          name=f"anc_ps{rep}_{w}",
                                          tag="anchor_ps", bufs=1)
                ohw_cl = {}
                c0 = 0
                for ci, csz in enumerate(zchunks):
                    g0 = w * T_W + c0
                    ohwc = ohw_pool.tile([P, csz, P], bf16,
                                         name=f"ohwc{rep}_{w}_{ci}",
                                         tag=f"ohwc{csz}",
                                         bufs=(5 if csz == 16 else 12))
                    nc.vector.tensor_tensor(
                        ohwc[:],
                        seg_sb[:, g0:g0 + csz, None].broadcast_to([P, csz, P]),
                        iota_sb[:, None, :].broadcast_to([P, csz, P]),
                        Alu.is_equal)
                    ohw_cl[ci] = ohwc
                    for tl in range(csz):
                        t = c0 + tl
                        nc.tensor.matmul(anchor_ps[:], ohwc[:, tl, :], nat_t(t),
                                         start=(t == 0), stop=(t == T_W - 1))
                    c0 += csz
                st["ohw_cl"] = ohw_cl

                out_sb = wnd_pool.tile([P, 2 * D], f32, name=f"osb{rep}_{w}",
                                       tag="out_sb")
                nc.vector.tensor_scalar(out_sb[:, D:2 * D], anchor_ps[:, 0:D],
                                        rcnt_sb[:, w:w + 1], None, Alu.mult)
                st["out_sb"] = out_sb

                # fv8 = fp8((anchor @ Wv.T)*16 + bv*16); transpose scratch and
                # fv output share one PSUM bank (f32 transposes from out_sb)
                tfv = sps_pool.tile([P, 4 * P], f32, name=f"tfv{rep}_{w}",
                                    tag="tfv", bufs=1)
                for db in range(2):
                    nc.tensor.transpose(tfv[:, db * P:(db + 1) * P],
                                        out_sb[:, D + db * P:D + (db + 1) * P],
                                        idf_sb[:])
                anchT = wnd_pool.tile([P, 2, P], bf16, name=f"anchT{rep}_{w}",
                                      tag="anchT")
                nc.any.tensor_copy(anchT[:], tfv[:, 0:2 * P])
                for db in range(2):
                    nc.tensor.matmul(tfv[:, 2 * P:4 * P], anchT[:, db, :],
                                     wvT_sb[:, db, :],
                                     start=(db == 0), stop=(db == 1))
                fv8 = wnd_pool.tile([P, D], fp8, name=f"fv8{rep}_{w}", tag="fv8")
                nc.vector.tensor_tensor(fv8[:], tfv[:, 2 * P:4 * P], bvb_sb[:],
                                        Alu.add)
                st["fv8"] = fv8
                return st

            def emit_pass23(rep, w, st, interject=None):
                nat_t = st["nat_t"]
                ohw_cl = st["ohw_cl"]
                fv8 = st["fv8"]
                out_sb = st["out_sb"]
                hsplit = st["hsplit"]
                if8_hv = st["if8_hv"]
                ohT_hv = st["ohT_hv"]

                def _pick(hv, n0):
                    for o0, tile in reversed(hv):
                        if n0 >= o0:
                            return o0, tile
                    raise AssertionError

                def if8_sl(uc):
                    n0 = uc * 512
                    o0, tile = _pick(if8_hv, n0)
                    return tile[:, n0 - o0:n0 - o0 + 512, :]

                def ohT_sl(uc):
                    n0 = uc * 512
                    o0, tile = _pick(ohT_hv, n0)
                    return tile[:, n0 - o0:n0 - o0 + 512]

                wsum_ps = sps_pool.tile([P, D + 1], f32,
                                        name=f"wsum{rep}_{w}", tag="wsum_ps",
                                        bufs=1)
                e_ps = sps_pool.tile([P, T_W], f32, name=f"e_ps{rep}_{w}",
                                     tag="e_ps", bufs=1)
                z_win = wnd_pool.tile([P, T_W], f32, name=f"zw{rep}_{w}",
                                      tag="z_win")

                zchunks = st["zchunks"]
                zstart = [sum(zchunks[:i]) for i in range(len(zchunks))]
                # chunk ci ready after u-chunk (zstart+csz-1)//4
                zc_after = {}
                for ci, csz in enumerate(zchunks):
                    zc_after.setdefault((zstart[ci] + csz - 1) // 4,
                                        []).append(ci)

                def z_wsum_chunk(ci):
                    csz = zchunks[ci]
                    c0 = zstart[ci]
                    c1 = c0 + csz
                    sp = zch_pool.tile([P, csz], f32, name=f"sp{rep}_{w}_{ci}",
                                       tag=f"zchs{csz}")
                    nc.scalar.activation(sp[:], e_ps[:, c0:c1], Act.Sigmoid)
                    sn = zch_pool.tile([P, csz], f32, name=f"sn{rep}_{w}_{ci}",
                                       tag=f"zchn{csz}")
                    nc.scalar.activation(sn[:], e_ps[:, c0:c1], Act.Sigmoid,
                                         scale=-1.0)
                    rn = zch_pool.tile([P, csz], f32, name=f"rn{rep}_{w}_{ci}",
                                       tag=f"zchr{csz}")
                    nc.vector.reciprocal(rn[:], sn[:])
                    nc.vector.tensor_tensor(z_win[:, c0:c1], sp[:], rn[:],
                                            Alu.mult)
                    ohzc = ohw_pool.tile([P, csz, P], bf16,
                                         name=f"ohzc{rep}_{w}_{ci}",
                                         tag=f"ohzc{csz}",
                                         bufs=(3 if csz == 16 else 5))
                    nc.vector.tensor_tensor(
                        ohzc[:], ohw_cl[ci][:],
                        z_win[:, c0:c1, None].broadcast_to([P, csz, P]),
                        Alu.mult)
                    for tl in range(csz):
                        t = c0 + tl
                        nc.tensor.matmul(wsum_ps[:], ohzc[:, tl, :], nat_t(t),
                                         start=(t == 0), stop=(t == T_W - 1))

                pairs = [(2 * p, 2 * p + 1) for p in range(UC // 2)]
                if UC % 2:
                    pairs.append((UC - 1,))
                for pi, ucs in enumerate(pairs):
                    width = 512 * len(ucs)
                    sgT = sgT_pool.tile([P, 2, width], bf16,
                                        name=f"sgT{rep}_{w}_{pi}",
                                        tag=f"sgT{len(ucs)}",
                                        bufs=(3 if len(ucs) == 2 else 2))
                    for h in range(2):
                        u_ps = ups_pool.tile([P, 1024], f32,
                                             name=f"u_ps{rep}_{w}_{pi}_{h}",
                                             tag="u_ps", bufs=2)
                        for j, uc in enumerate(ucs):
                            sl = slice(j * 512, (j + 1) * 512)
                            nc.tensor.matmul(
                                u_ps[:, sl], wu8_sb[:, h, :, :],
                                if8_sl(uc).rearrange("p n k -> p k n"),
                                start=True, stop=False, perf_mode=DR)
                            nc.tensor.matmul(
                                u_ps[:, sl], fv8[:, h * P:(h + 1) * P],
                                ohT_sl(uc),
                                start=False, stop=True)
                        nc.scalar.activation(sgT[:, h, :], u_ps[:, 0:width],
                                             Act.Sigmoid, scale=0.0625)
                    for j, uc in enumerate(ucs):
                        for t4 in range(4):
                            t = uc * 4 + t4
                            for h in range(2):
                                nc.tensor.matmul(
                                    e_ps[:, t:t + 1],
                                    sgT[:, h, j * 512 + t4 * P:
                                        j * 512 + (t4 + 1) * P],
                                    web_sb[:, h:h + 1],
                                    start=(h == 0), stop=(h == 1))
                        for ci in zc_after.get(uc, ()):
                            z_wsum_chunk(ci)
                        if interject is not None and uc == UC - 5:
                            interject()

                den = col_pool.tile([P, 1], f32, name=f"den{rep}_{w}", tag="col")
                nc.vector.tensor_scalar(den[:], wsum_ps[:, D:D + 1], 1e-30,
                                        None, Alu.max)
                rden = col_pool.tile([P, 1], f32, name=f"rden{rep}_{w}",
                                     tag="col")
                nc.vector.reciprocal(rden[:], den[:])
                nc.vector.tensor_scalar(out_sb[:, 0:D], wsum_ps[:, 0:D],
                                        rden[:], None, Alu.mult)
                nc.sync.dma_start(out_dram[w], out_sb[:])

            def body(rep):
                st0 = emit_loads(rep, 0)
                emit_pass1(rep, 0, st0)
                st1 = emit_loads(rep, 1)

                def _splice():
                    emit_pass1(rep, 1, st1)

                emit_pass23(rep, 0, st0, interject=_splice)
                emit_pass23(rep, 1, st1)

            if loop_repeat is not None:
                with tc.For_i(0, loop_repeat, 1):
                    body("L")
            else:
                for rep in range(repeat):
                    body(rep)

    return nc


def _prepare(ifeat, Wu, Wv, bv, we, seg_ids):
    """Host-side shard + pad + layout. Returns (T_W, in_maps)."""
    ifeat = np.asarray(ifeat, dtype=np.float32)
    Wu = np.asarray(Wu, dtype=np.float32)
    Wv = np.asarray(Wv, dtype=np.float32)
    bv = np.asarray(bv, dtype=np.float32)
    we = np.asarray(we, dtype=np.float32)
    seg_ids = np.asarray(seg_ids)

    bounds = np.searchsorted(
        seg_ids, np.arange(0, B + 1, SEGS_PER_WINDOW), side="left")
    n_w = np.diff(bounds)
    T_W = max(4, int(-(-int(n_w.max()) // P)))
    T_W = ((T_W + 3) // 4) * 4
    NT = W_PER_CORE * T_W

    # wu8p[ki, (h, ko, m)] = Wu[h*128+m, ko*128+ki] * 16
    wu8p = np.ascontiguousarray(
        (Wu * 16.0).reshape(2, P, 2, P).transpose(3, 0, 2, 1)
        .reshape(P, 512)).astype(F8)
    wvT16 = np.ascontiguousarray(Wv.T * 16.0).reshape(2, P, D)
    # cbf[p] = [wvT16 (k d: 512), web2 (2), idb (128), iota (128)]
    cbf = np.concatenate([
        wvT16.transpose(1, 0, 2).reshape(P, 512),
        np.ascontiguousarray(we.reshape(2, P).T),
        np.eye(P, dtype=np.float32),
        np.tile(np.arange(P, dtype=np.float32), (P, 1)),
    ], axis=1).astype(BF)
    bvb16 = np.tile(bv * 16.0, (P, 1)).astype(np.float32)

    in_maps = []
    for c in range(N_CORES):
        nat = np.zeros((NT * P, D + 1), dtype=np.float32)
        nat[:, D] = 1.0
        seg = np.full((NT * P,), 500.0, dtype=np.float32)
        rcnt = np.zeros((P, W_PER_CORE), dtype=np.float32)
        for wl in range(W_PER_CORE):
            w = c * W_PER_CORE + wl
            lo, hi = bounds[w], bounds[w + 1]
            base = wl * T_W * P
            nat[base:base + (hi - lo), 0:D] = ifeat[lo:hi]
            seg_loc = seg_ids[lo:hi].astype(np.int64) - w * SEGS_PER_WINDOW
            seg[base:base + (hi - lo)] = seg_loc.astype(np.float32)
            cnt = np.bincount(seg_loc, minlength=P).astype(np.float32)
            rcnt[:, wl] = 1.0 / np.maximum(cnt, 1.0)
        natb = nat.astype(BF).reshape(NT, P, D + 1)
        natp = np.ascontiguousarray(natb.transpose(1, 0, 2))     # [P, NT, 257]
        x8 = nat[:, 0:D].astype(F8)                              # [n, fi]
        ift8p = np.ascontiguousarray(
            x8.reshape(NT * P, 2, P).transpose(2, 0, 1))         # [ki, n, ko]
        ohT8p = (seg[None, :] ==
                 np.arange(P, dtype=np.float32)[:, None]).astype(F8)
        segp = np.ascontiguousarray(seg.reshape(NT, P).T)        # [P, NT]
        cf32 = np.concatenate([bvb16, rcnt, segp,
                               np.eye(P, dtype=np.float32)],
                              axis=1).astype(np.float32)
        in_maps.append({
            "natp": natp, "ift8p": ift8p, "ohT8p": ohT8p,
            "cbf": cbf, "cf32": cf32, "wu8p": wu8p,
        })
    return T_W, in_maps


_LAST = {}


def _run(ifeat, Wu, Wv, bv, we, seg_ids, trace=False):
    from concourse.bass_utils import run_bass_kernel_spmd

    T_W, in_maps = _prepare(ifeat, Wu, Wv, bv, we, seg_ids)
    nc = _build(T_W)
    _split_sync_waits(nc)
    res = run_bass_kernel_spmd(nc, in_maps, list(range(N_CORES)), trace=trace)
    _LAST["res"] = res
    _LAST["T_W"] = T_W
    _LAST["nc"] = nc
    _LAST["in_maps"] = in_maps

    out = np.empty((B, 2 * D), dtype=np.float32)
    for c in range(N_CORES):
        core_out = res.results[c]["out"]  # [W_PER_CORE, P, 2D]
        for wl in range(W_PER_CORE):
            w = c * W_PER_CORE + wl
            out[w * SEGS_PER_WINDOW:(w + 1) * SEGS_PER_WINDOW, :] = core_out[wl]
    return out


def kernel(ifeat, Wu, Wv, bv, we, seg_ids):
    return _run(ifeat, Wu, Wv, bv, we, seg_ids, trace=False)
